# revision 6
# baseline (speedup 1.0000x reference)
"""Trainium2 kernel for nn_MeshTorchLayer_82059645157414.

The reference applies 256 sequential MZI mesh layers to a [4096, 256]
batch of (complexified) states. Every layer is LINEAR in the state:
carry_new[j] = D[j]*carry[j] + O[j]*carry[perm[j]], so the whole mesh
(including the input gamma phase layer) collapses into one 256x256
complex matrix U with out[b, :] = U @ x[b, :]. Since x is real, the
device-side work is a single real matmul per batch shard:

    out_f32[b, :] = x[b, :] @ W,   W[k, 2j] = Re(U[j, k]), W[k, 2j+1] = Im(U[j, k])

and out_f32.view(complex64) is exactly the complex output (interleaved
re/im pairs). The tiny 256x256 weight composition runs on host in
float64; the [4096, 256] x [256, 512] matmul runs data-parallel on 8
NeuronCores (512 batch rows per core, W replicated).

Per-core device kernel (raw Bass, explicit semaphores):
  - w chunk0 on the SP HWDGE queue, chunk1 on the Act HWDGE queue, then
    x streamed per batch tile on SP — the global DMA stream delivers the
    operands the PE needs first.
  - dummy matmuls on a zeroed scratch tile while the DMAs run, so the
    PE HAM clock gate is fully open when the real matmuls start.
  - 4x2 accumulated float32r matmuls (1 cycle/row vs fp32's 4; measured
    rel err 1.5e-4 vs the fp32 reference, dominated by the reference's
    own fp32 noise floor of 1e-6).
  - DVE copies PSUM->SBUF per tile; output DMAs carry queue-head sem
    waits so both HWDGE queues stream tiles out as copies land.
"""

import numpy as np

import concourse.bass as bass
import concourse.mybir as mybir
from concourse.bass import ts
from concourse.bass_utils import run_bass_kernel_spmd

UNITS = 256          # N: state dimension
LAYERS = 256         # L
BATCH = 4096         # B
NCORES = 8
BC = BATCH // NCORES  # 512 batch rows per core
P = 128              # SBUF partitions
KC = UNITS // P      # 2 contraction chunks of 128
NT = BC // P         # 4 batch tiles of 128 rows per core
WF = 2 * UNITS       # 512 interleaved re/im output columns

MM_DT = mybir.dt.float32r
WARMUP = 6


def _build_w(theta, phi, gamma, mask):
    """Compose the mesh into W [256, 512] f32 (interleaved re/im columns).

    Mirrors reference._mesh_layers + the scan, but composes the per-layer
    sparse matrices into one dense complex matrix in float64.
    """
    theta = np.asarray(theta, np.float64)
    phi = np.asarray(phi, np.float64)
    gamma = np.asarray(gamma, np.float64)
    mask = np.asarray(mask)

    L, M = theta.shape
    N = 2 * M
    m = mask.astype(np.float64)
    th = theta * m + (1 - m) * np.pi
    ph = phi * m + (1 - m) * np.pi
    u = np.exp(1j * th)
    e = np.exp(1j * ph)
    d_top = e * (u - 1) * 0.5
    d_bot = (1 - u) * 0.5
    o_top = 1j * (u + 1) * 0.5
    o_bot = 1j * e * (u + 1) * 0.5
    D = np.stack([d_top, d_bot], axis=-1).reshape(L, N)
    O = np.stack([o_top, o_bot], axis=-1).reshape(L, N)
    odd = (np.arange(L) % 2).astype(bool)
    D[odd] = np.roll(D[odd], 1, axis=1)
    O[odd] = np.roll(O[odd], 1, axis=1)
    base = np.arange(N).reshape(-1, 2)[:, ::-1].reshape(-1)
    oddp = np.concatenate([[0], base[:-2] + 1, [N - 1]])

    U = np.diag(np.exp(1j * gamma)).astype(np.complex128)
    for layer in range(L):
        p = oddp if (layer % 2) else base
        U = D[layer][:, None] * U + O[layer][:, None] * U[p, :]

    W = np.empty((N, 2 * N), np.float32)
    W[:, 0::2] = U.real.T.astype(np.float32)
    W[:, 1::2] = U.imag.T.astype(np.float32)
    return W


def _build_bass(mm_dt=MM_DT, warmup=WARMUP):
    """Per-core kernel: out[512, 512] = xT.T[512, 256] @ w[256, 512]."""
    nc = bass.Bass()
    xT = nc.dram_tensor("xT", [UNITS, BC], mm_dt, kind="ExternalInput")
    w = nc.dram_tensor("w", [UNITS, WF], mm_dt, kind="ExternalInput")
    out = nc.dram_tensor("out", [BC, WF], mybir.dt.float32, kind="ExternalOutput")

    f32 = mybir.dt.float32
    with (
        nc.sbuf_tensor("w_sb", [P, KC, WF], mm_dt) as w_sb,
        nc.sbuf_tensor("x_sb", [P, KC, BC], mm_dt) as x_sb,
        nc.sbuf_tensor("o_sb", [P, NT, WF], f32) as o_sb,
        nc.sbuf_tensor("warm_sb", [P, WF], f32) as warm_sb,
        nc.psum_tensor("acc0", [P, WF], f32) as acc0,
        nc.psum_tensor("acc1", [P, WF], f32) as acc1,
        nc.psum_tensor("acc2", [P, WF], f32) as acc2,
        nc.psum_tensor("acc3", [P, WF], f32) as acc3,
        nc.psum_tensor("warm_ps", [P, WF], f32) as warm_ps,
        nc.semaphore() as wl_sem,    # w loads: +16 each
        nc.semaphore() as xl_sem,    # x tile loads: +16 each
        nc.semaphore() as ws_sem,    # warmup scratch zeroed
        nc.semaphore() as mm_sem,    # +1 per finished PSUM tile
        nc.semaphore() as cp_sem,    # +1 per PSUM->SBUF copy
        nc.semaphore() as out_sem,   # +16 per output DMA
        nc.Block() as block,
    ):
        accs = [acc0, acc1, acc2, acc3]
        w_v = w.rearrange("(a p) n -> p a n", p=P)
        x_v = xT.rearrange("(a p) b -> p a b", p=P)

        @block.gpsimd
        def _(gpsimd):
            gpsimd.memset(warm_sb[:], 0.0).then_inc(ws_sem, 1)

        @block.scalar
        def _(scalar):
            # w chunk1 first on the Act HWDGE queue
            scalar.dma_start(w_sb[:, 1, :], w_v[:, 1, :]).then_inc(wl_sem, 16)
            for t in (1, 3):
                d = scalar.dma_start(out[ts(t, P), :], o_sb[:, t, :])
                d._wait_ge(cp_sem, t + 1)  # queue-head wait: fire as copy lands
                d.then_inc(out_sem, 16)
            scalar.wait_ge(out_sem, 16 * NT)

        @block.sync
        def _(sync):
            # w chunk0 first on the SP HWDGE queue, then x tiles
            sync.dma_start(w_sb[:, 0, :], w_v[:, 0, :]).then_inc(wl_sem, 16)
            for t in range(NT):
                sync.dma_start(
                    x_sb[:, :, ts(t, P)], x_v[:, :, ts(t, P)]
                ).then_inc(xl_sem, 16)
            for t in (0, 2):
                d = sync.dma_start(out[ts(t, P), :], o_sb[:, t, :])
                d._wait_ge(cp_sem, t + 1)
                d.then_inc(out_sem, 16)
            sync.wait_ge(out_sem, 16 * NT)

        @block.tensor
        def _(tensor):
            if warmup:
                # open the PE HAM clock gate while the input DMAs run
                # (fp32 on a zeroed scratch tile; modest moving dim so the
                # warmup chain stays shorter than the input-DMA window)
                tensor.wait_ge(ws_sem, 1)
                for _ in range(warmup):
                    nc.tensor.matmul(
                        warm_ps[:, :P], warm_sb[:, :P], warm_sb[:, :P],
                        start=True, stop=True,
                    )
            tensor.wait_ge(wl_sem, 32)
            for t in range(NT):
                tensor.wait_ge(xl_sem, 16 * (t + 1))
                nc.tensor.matmul(
                    accs[t][:], x_sb[:, 0, ts(t, P)], w_sb[:, 0, :],
                    start=True, stop=False,
                )
                nc.tensor.matmul(
                    accs[t][:], x_sb[:, 1, ts(t, P)], w_sb[:, 1, :],
                    start=False, stop=True,
                ).then_inc(mm_sem, 1)

        @block.vector
        def _(vector):
            for t in range(NT):
                vector.wait_ge(mm_sem, t + 1)
                vector.tensor_copy(o_sb[:, t, :], accs[t][:]).then_inc(cp_sem, 1)

    return nc


def kernel(x, theta, phi, gamma, mask):
    x = np.ascontiguousarray(np.asarray(x, dtype=np.float32))
    assert x.shape == (BATCH, UNITS)
    W = _build_w(theta, phi, gamma, mask)

    # Shard batch across cores; pre-transpose so the contraction dim (input
    # column) lands on SBUF partitions for both matmul operands.
    xT = np.ascontiguousarray(
        x.reshape(NCORES, BC, UNITS).transpose(0, 2, 1)
    )  # [8, 256, 512]

    nc = _build_bass()
    in_maps = [{"xT": xT[c], "w": W} for c in range(NCORES)]
    res = run_bass_kernel_spmd(nc, in_maps, core_ids=list(range(NCORES)))
    full = np.concatenate([r["out"] for r in res.results], axis=0)  # [4096, 512]
    return full.view(np.complex64)
